# revision 39
# baseline (speedup 1.0000x reference)
"""ExternalAttention Trainium2 Bass kernel (transposed bf16 dataflow).

Math (per batch b, with N = H*W = 4096 tokens, C = 512, K = 64):
    x      = inputs @ w1 + b1          [N, C]
    logits = x @ m0                    [N, K]
    attn   = softmax(logits, axis=N)   (trailing L1-norm divides by 1+1e-9 ->
                                        skipped; max-shift skipped: logits O(1))
    y      = attn @ m1 @ w2            [N, C]
    out    = relu(BN_affine(y) + inputs)

Host-side folds (tiny C x C / C x K matrices):
    wm    = [w1 @ m0 | 0]                       [C, K+1]  (b1 @ m0 is constant
            per softmax column -> softmax-invariant, dropped; the zero column
            gives exp(0)=1, a ones-row that injects the BN shift via mm2)
    scale = gamma / sqrt(bn_var + eps); shift = beta - bn_mean * scale
    w2m   = [m1 @ (w2 * scale) ; shift]         [K+1, C]
    => out = relu(colsoftmax(x @ wm) @ w2m + x)

Device dataflow (per core, 2 batches, data-parallel over B=16 on 8 cores).
The host ships x TRANSPOSED per batch (xT [C, N], bf16) so the kernel never
transposes on the PE; everything runs in [c, n] layout and the host
transposes the bf16 output back.
    - mm1:  logitsT[65, n512] = sum_ct wm[ct]^T @ xT[ct], chunk-pairs with
      bank-interleaved psum accumulation
    - ACT exp psum -> attn [65, N] bf16 with accumulated row sums; softmax
      row scale 1/sum is folded into a per-batch bf16 copy of w2m (scaling
      w2m rows == scaling attn rows), applied as an ACT per-partition scale
    - mm2:  deltaT[ct] [128, n512] = w2m_b[:, ct]^T @ attn  (single bf16
      matmul, psum-write-bound ~373ns)
    - residual add: some tiles on the PE (ident matmul accumulate) with ACT
      relu from psum; the rest on DVE (psum+xT -> bf16) with cheap 16-bit
      in-place DVE relu
    - two-batch software pipeline: b0's mm2 epilogue interleaves with b1's
      mm1 so the PE never waits on b1's softmax reduction
Queues: SP carries only the 3 tiny constant loads (the SP engine broadcasts
tile-clock semaphores; bulk DMA issue there stalls every engine), x loads
ride SP+gpsimd split by ctile, stores ride gpsimd.
"""

import os
import sys
from contextlib import ExitStack

import numpy as np
import ml_dtypes

for _p in ("/opt/trn_rl_repo", os.path.expanduser("~/.axon_site/_ro/trn_rl_repo")):
    if os.path.isdir(_p) and _p not in sys.path:
        sys.path.insert(0, _p)

import concourse.bass as bass
import concourse.mybir as mybir
import concourse.tile as tile
from concourse import bacc
from concourse.bass import ts
from concourse.bass_utils import run_bass_kernel_spmd

B, H, W, C, K = 16, 64, 64, 512, 64
N = H * W  # 4096 tokens
BN_EPS = 1e-3
NCORES = 8
BPC = B // NCORES  # batches per core = 2

F32 = mybir.dt.float32
BF16 = mybir.dt.bfloat16
BF16_NP = ml_dtypes.bfloat16

CT = 4          # channel tiles of 128
NQ = 8          # token chunks of 512 per batch
QW = 512        # chunk width

_cached_nc = None


def _build_nc() -> bass.Bass:
    nc = bacc.Bacc(None, target_bir_lowering=False, debug=False)
    x = nc.dram_tensor("x", [BPC, C, N], BF16, kind="ExternalInput")
    # wm pre-swizzled on host to [p, ct*(K+1)] so the load is one contiguous
    # run per partition (128 descriptors instead of 512)
    wm = nc.dram_tensor("wm", [128, CT * (K + 1)], BF16, kind="ExternalInput")
    w2m = nc.dram_tensor("w2m", [K + 1, C], F32, kind="ExternalInput")
    ident = nc.dram_tensor("ident", [128, 128], BF16, kind="ExternalInput")
    y = nc.dram_tensor("y", [BPC, C, N], BF16, kind="ExternalOutput")

    with tile.TileContext(nc) as tc, ExitStack() as ctx:
        const = ctx.enter_context(tc.tile_pool(name="const", bufs=1))
        xt_pool = ctx.enter_context(tc.tile_pool(name="xt", bufs=2 * CT))
        attn_pool = ctx.enter_context(tc.tile_pool(name="attn", bufs=2))
        out_pool = ctx.enter_context(tc.tile_pool(name="out", bufs=2))
        small = ctx.enter_context(tc.tile_pool(name="small", bufs=4))

        wm_sb = const.tile([128, CT, K + 1], BF16)   # wm[ct*128+p, k]
        w2m_sb = const.tile([K + 1, C], F32)         # fp32 master
        ident_sb = const.tile([128, 128], BF16)

        xbs = [x[b].rearrange("(ct p) n -> ct p n", p=128) for b in range(BPC)]
        ybs = [y[b].rearrange("(ct p) n -> ct p n", p=128) for b in range(BPC)]

        xts, attns, sums_t, w2mb_t = [], [], [], []

        def load_batch(b):
            xts.append([xt_pool.tile([128, N], BF16, tag="xt",
                                     name=f"xt{b}_{ct}") for ct in range(CT)])
            # b0 gets a fine-grained front so mm1 can start earlier; issue
            # is split between the SP and gpsimd queues (ct0/1 vs ct2/3)
            if b == 0:
                bounds = (0, 1024, 2048, N)
            else:
                bounds = (0, 2048, N)
            for lo, hi in zip(bounds, bounds[1:]):
                for ct in range(CT):
                    eng = nc.gpsimd if ct >= 2 else nc.sync
                    eng.dma_start(out=xts[b][ct][:, lo:hi],
                                  in_=xbs[b][ct][:, lo:hi])
            attns.append(attn_pool.tile([K + 1, N], BF16, tag="attn",
                                        name=f"attn{b}"))
            sums_t.append(small.tile([K + 1, NQ], F32, tag="sums",
                                     name=f"sums{b}"))
            w2mb_t.append(small.tile([K + 1, C], BF16, tag="w2mb",
                                     name=f"w2mb{b}"))

        def mm1_pair(l_psum, b, q0):
            # two chunks with bank-interleaved accumulation: consecutive PE
            # instructions alternate psum banks, hiding the RMW drain
            q1 = q0 + 1
            p_a = l_psum.tile([K + 1, QW], F32, tag="l")
            p_b = l_psum.tile([K + 1, QW], F32, tag="l")
            for ct in range(CT):
                for p_l, q in ((p_a, q0), (p_b, q1)):
                    nc.tensor.matmul(
                        p_l,
                        lhsT=wm_sb[:, ct],
                        rhs=xts[b][ct][:, ts(q, QW)],
                        start=(ct == 0),
                        stop=(ct == CT - 1),
                    )
            # exp straight from psum; row K is exp(0)=1 (ones row);
            # per-chunk row sums accumulate into sums[:, q]
            for p_l, q in ((p_a, q0), (p_b, q1)):
                nc.scalar.activation(
                    out=attns[b][:, ts(q, QW)], in_=p_l,
                    func=mybir.ActivationFunctionType.Exp,
                    accum_out=sums_t[b][:, q:q + 1],
                )

        def softmax_finish(b):
            total = small.tile([K + 1, 1], F32, tag="total")
            nc.vector.reduce_sum(out=total, in_=sums_t[b],
                                 axis=mybir.AxisListType.X)
            rsum = small.tile([K + 1, 1], F32, tag="rsum")
            nc.vector.reciprocal(out=rsum, in_=total)
            # ones-row (BN shift) must not be normalized
            nc.vector.memset(rsum[K:K + 1], 1.0)
            # fold softmax 1/sum into the mm2 weights: w2m_b = rsum * w2m
            # (ACT per-partition scale; DVE's tensor_scalar here costs 3us)
            nc.scalar.activation(
                out=w2mb_t[b], in_=w2m_sb,
                func=mybir.ActivationFunctionType.Copy, scale=rsum)

        def mm2_ctile(y_psum, b, ct, last=False):
            out_t = out_pool.tile([128, N], BF16, tag="out",
                                  name=f"out{b}_{ct}")
            for q in range(NQ):
                p_y = y_psum.tile([128, QW], F32, tag="y")
                # engine balance: b0's epilogue hides behind b1's mm1 on the
                # PE, so b0 leans on DVE; b1's tail splits PE/ACT vs DVE
                pe_add = q in ((0, 4), (0, 2, 4, 6))[b]
                nc.tensor.matmul(
                    p_y,
                    lhsT=w2mb_t[b][:, ts(ct, 128)],
                    rhs=attns[b][:, ts(q, QW)],
                    start=True, stop=not pe_add,
                )
                if pe_add:
                    # residual add for free on the PE: psum += I @ xT;
                    # relu straight from psum on ACT
                    nc.tensor.matmul(
                        p_y,
                        lhsT=ident_sb,
                        rhs=xts[b][ct][:, ts(q, QW)],
                        start=False, stop=True,
                    )
                    nc.scalar.activation(
                        out=out_t[:, ts(q, QW)], in_=p_y,
                        func=mybir.ActivationFunctionType.Relu,
                    )
                else:
                    # DVE residual add psum+xT -> bf16, then cheap 16-bit
                    # in-place relu (DVE mostly; ACT takes one for balance)
                    nc.vector.scalar_tensor_tensor(
                        out=out_t[:, ts(q, QW)],
                        in0=p_y, scalar=1.0, in1=xts[b][ct][:, ts(q, QW)],
                        op0=mybir.AluOpType.mult, op1=mybir.AluOpType.add,
                    )
                    if q == 7 and b == 0:
                        nc.scalar.activation(
                            out=out_t[:, ts(q, QW)], in_=out_t[:, ts(q, QW)],
                            func=mybir.ActivationFunctionType.Relu,
                        )
                    else:
                        nc.vector.tensor_scalar_max(
                            out_t[:, ts(q, QW)], out_t[:, ts(q, QW)], 0.0)
                if q == NQ // 2 - 1 or q == NQ - 1:
                    # store in halves; the very last half goes out in
                    # quarters so the drain tail after the final relu is
                    # as short as possible
                    lo = 0 if q == NQ // 2 - 1 else N // 2
                    if last and q == NQ - 1:
                        for o in (0, N // 4):
                            nc.gpsimd.dma_start(
                                out=ybs[b][ct][:, lo + o:lo + o + N // 4],
                                in_=out_t[:, lo + o:lo + o + N // 4])
                    else:
                        nc.gpsimd.dma_start(
                            out=ybs[b][ct][:, lo:lo + N // 2],
                            in_=out_t[:, lo:lo + N // 2])

        # preload the ACT exp table off the critical path via a dummy exp
        dummy = small.tile([1, 1], F32, tag="dummy")
        nc.vector.memset(dummy, 0.0)
        nc.scalar.activation(out=dummy, in_=dummy,
                             func=mybir.ActivationFunctionType.Exp)
        # constants first on SP so mm1 can start the moment x arrives
        nc.sync.dma_start(out=wm_sb.rearrange("p ct k -> p (ct k)"),
                          in_=wm[:, :])
        nc.sync.dma_start(out=w2m_sb, in_=w2m[:, :])
        nc.sync.dma_start(out=ident_sb, in_=ident[:, :])
        load_batch(0)
        load_batch(1)

        with tc.tile_pool(name="lps", bufs=4, space="PSUM") as l_psum, \
             tc.tile_pool(name="yps", bufs=4, space="PSUM") as y_psum:
            for qq in range(NQ // 2):
                mm1_pair(l_psum, 0, 2 * qq)
            softmax_finish(0)
            # b1 mm1 interleaved with b0's mm2 epilogue. The first round
            # leads with b0's mm2 (b1's loads haven't landed yet and the
            # in-order PE must not stall on them); later rounds lead with
            # the b1 pair, which fills the PE while DVE/ACT drain the
            # previous mm2 ctile's psum consumers.
            mm2_ctile(y_psum, 0, 0)
            for ct in range(1, CT):
                mm1_pair(l_psum, 1, 2 * (ct - 1))
                mm2_ctile(y_psum, 0, ct)
            mm1_pair(l_psum, 1, 2 * (CT - 1))
            softmax_finish(1)
            for ct in range(CT):
                mm2_ctile(y_psum, 1, ct, last=(ct == CT - 1))

    nc.finalize()
    return nc


def _get_nc() -> bass.Bass:
    global _cached_nc
    if _cached_nc is None:
        _cached_nc = _build_nc()
    return _cached_nc


def _fold_weights(w1, m0, m1, w2, gamma, beta, bn_mean, bn_var):
    w1 = np.asarray(w1, np.float64)
    m0 = np.asarray(m0, np.float64)
    m1 = np.asarray(m1, np.float64)
    w2 = np.asarray(w2, np.float64)
    gamma = np.asarray(gamma, np.float64)
    beta = np.asarray(beta, np.float64)
    bn_mean = np.asarray(bn_mean, np.float64)
    bn_var = np.asarray(bn_var, np.float64)

    wm_aug = np.zeros((C, K + 1), np.float32)
    wm_aug[:, :K] = (w1 @ m0).astype(np.float32)  # col K stays 0 -> ones row
    scale = gamma / np.sqrt(bn_var + BN_EPS)
    w2m_aug = np.zeros((K + 1, C), np.float32)
    w2m_aug[:K] = (m1 @ (w2 * scale[None, :])).astype(np.float32)
    w2m_aug[K] = (beta - bn_mean * scale).astype(np.float32)  # shift row
    return wm_aug, w2m_aug


def _run(inputs_np: dict, trace: bool = False):
    nc = _get_nc()
    inp = np.asarray(inputs_np["inputs"], np.float32)
    wm_aug, w2m_aug = _fold_weights(
        inputs_np["w1"], inputs_np["m0"], inputs_np["m1"], inputs_np["w2"],
        inputs_np["gamma"], inputs_np["beta"],
        inputs_np["bn_mean"], inputs_np["bn_var"],
    )
    # per-batch transposed bf16 inputs: [B, C, N]
    xT = np.ascontiguousarray(
        inp.reshape(B, N, C).astype(BF16_NP).transpose(0, 2, 1))
    # swizzle wm to [p, ct*(K+1)]: row p holds wm[ct*128+p, :] for ct=0..3
    wm_bf = np.ascontiguousarray(
        wm_aug.astype(BF16_NP).reshape(CT, 128, K + 1)
        .transpose(1, 0, 2).reshape(128, CT * (K + 1)))
    eye = np.eye(128, dtype=BF16_NP)
    in_maps = [
        {
            "x": xT[i * BPC:(i + 1) * BPC],
            "wm": wm_bf,
            "w2m": w2m_aug,
            "ident": eye,
        }
        for i in range(NCORES)
    ]
    res = run_bass_kernel_spmd(nc, in_maps, core_ids=list(range(NCORES)),
                               trace=trace)
    out = np.concatenate([r["y"] for r in res.results], axis=0)  # [B, C, N]
    out = out.transpose(0, 2, 1).astype(np.float32)
    return np.ascontiguousarray(out).reshape(B, H, W, C), res


def kernel(**inputs) -> np.ndarray:
    out, _ = _run(inputs, trace=False)
    return out


# revision 41
# speedup vs baseline: 1.0630x; 1.0630x over previous
"""ExternalAttention Trainium2 Bass kernel (transposed bf16 dataflow).

Math (per batch b, with N = H*W = 4096 tokens, C = 512, K = 64):
    x      = inputs @ w1 + b1          [N, C]
    logits = x @ m0                    [N, K]
    attn   = softmax(logits, axis=N)   (trailing L1-norm divides by 1+1e-9 ->
                                        skipped; max-shift skipped: logits O(1))
    y      = attn @ m1 @ w2            [N, C]
    out    = relu(BN_affine(y) + inputs)

Host-side folds (tiny C x C / C x K matrices):
    wm    = [w1 @ m0 | 0]                       [C, K+1]  (b1 @ m0 is constant
            per softmax column -> softmax-invariant, dropped; the zero column
            gives exp(0)=1, a ones-row that injects the BN shift via mm2)
    scale = gamma / sqrt(bn_var + eps); shift = beta - bn_mean * scale
    w2m   = [m1 @ (w2 * scale) ; shift]         [K+1, C]
    => out = relu(colsoftmax(x @ wm) @ w2m + x)

Device dataflow (per core, 2 batches, data-parallel over B=16 on 8 cores).
The host ships x TRANSPOSED per batch (xT [C, N], bf16) so the kernel never
transposes on the PE; everything runs in [c, n] layout and the host
transposes the bf16 output back.
    - mm1:  logitsT[65, n512] = sum_ct wm[ct]^T @ xT[ct], chunk-pairs with
      bank-interleaved psum accumulation
    - ACT exp psum -> attn [65, N] bf16 with accumulated row sums; softmax
      row scale 1/sum is folded into a per-batch bf16 copy of w2m (scaling
      w2m rows == scaling attn rows), applied as an ACT per-partition scale
    - mm2:  deltaT[ct] [128, n512] = w2m_b[:, ct]^T @ attn  (single bf16
      matmul, psum-write-bound ~373ns)
    - residual add: some tiles on the PE (ident matmul accumulate) with ACT
      relu from psum; the rest on DVE (psum+xT -> bf16) with cheap 16-bit
      in-place DVE relu
    - two-batch software pipeline: b0's mm2 epilogue interleaves with b1's
      mm1 so the PE never waits on b1's softmax reduction
Queues: SP carries only the 3 tiny constant loads (the SP engine broadcasts
tile-clock semaphores; bulk DMA issue there stalls every engine), x loads
ride SP+gpsimd split by ctile, stores ride gpsimd.
"""

import os
import sys
from contextlib import ExitStack

import numpy as np
import ml_dtypes

for _p in ("/opt/trn_rl_repo", os.path.expanduser("~/.axon_site/_ro/trn_rl_repo")):
    if os.path.isdir(_p) and _p not in sys.path:
        sys.path.insert(0, _p)

import concourse.bass as bass
import concourse.mybir as mybir
import concourse.tile as tile
from concourse import bacc
from concourse.bass import ts
from concourse.bass_utils import run_bass_kernel_spmd

B, H, W, C, K = 16, 64, 64, 512, 64
N = H * W  # 4096 tokens
BN_EPS = 1e-3
NCORES = 8
BPC = B // NCORES  # batches per core = 2

F32 = mybir.dt.float32
BF16 = mybir.dt.bfloat16
BF16_NP = ml_dtypes.bfloat16

CT = 4          # channel tiles of 128
NQ = 8          # token chunks of 512 per batch
QW = 512        # chunk width

_cached_nc = None


def _build_nc() -> bass.Bass:
    nc = bacc.Bacc(None, target_bir_lowering=False, debug=False)
    x = nc.dram_tensor("x", [BPC, C, N], BF16, kind="ExternalInput")
    # wm pre-swizzled on host to [p, ct*(K+1)] so the load is one contiguous
    # run per partition (128 descriptors instead of 512)
    wm = nc.dram_tensor("wm", [128, CT * (K + 1)], BF16, kind="ExternalInput")
    w2m = nc.dram_tensor("w2m", [K + 1, C], F32, kind="ExternalInput")
    ident = nc.dram_tensor("ident", [128, 128], BF16, kind="ExternalInput")
    y = nc.dram_tensor("y", [BPC, C, N], BF16, kind="ExternalOutput")

    with tile.TileContext(nc) as tc, ExitStack() as ctx:
        const = ctx.enter_context(tc.tile_pool(name="const", bufs=1))
        xt_pool = ctx.enter_context(tc.tile_pool(name="xt", bufs=2 * CT))
        attn_pool = ctx.enter_context(tc.tile_pool(name="attn", bufs=2))
        out_pool = ctx.enter_context(tc.tile_pool(name="out", bufs=2))
        small = ctx.enter_context(tc.tile_pool(name="small", bufs=4))

        wm_sb = const.tile([128, CT, K + 1], BF16)   # wm[ct*128+p, k]
        w2m_sb = const.tile([K + 1, C], F32)         # fp32 master
        ident_sb = const.tile([128, 128], BF16)

        xbs = [x[b].rearrange("(ct p) n -> ct p n", p=128) for b in range(BPC)]
        ybs = [y[b].rearrange("(ct p) n -> ct p n", p=128) for b in range(BPC)]

        xts, attns, sums_t, w2mb_t = [], [], [], []

        def load_batch(b):
            xts.append([xt_pool.tile([128, N], BF16, tag="xt",
                                     name=f"xt{b}_{ct}") for ct in range(CT)])
            # b0 gets a fine-grained front so mm1 can start earlier; issue
            # is split between the SP and gpsimd queues (ct0/1 vs ct2/3)
            if b == 0:
                bounds = (0, 1024, 2048, N)
            else:
                bounds = (0, 2048, N)
            for lo, hi in zip(bounds, bounds[1:]):
                for ct in range(CT):
                    eng = nc.gpsimd if (b == 0 and ct >= 2) else nc.sync
                    eng.dma_start(out=xts[b][ct][:, lo:hi],
                                  in_=xbs[b][ct][:, lo:hi])
            attns.append(attn_pool.tile([K + 1, N], BF16, tag="attn",
                                        name=f"attn{b}"))
            sums_t.append(small.tile([K + 1, NQ], F32, tag="sums",
                                     name=f"sums{b}"))
            w2mb_t.append(small.tile([K + 1, C], BF16, tag="w2mb",
                                     name=f"w2mb{b}"))

        def mm1_pair(l_psum, b, q0):
            # two chunks with bank-interleaved accumulation: consecutive PE
            # instructions alternate psum banks, hiding the RMW drain
            q1 = q0 + 1
            p_a = l_psum.tile([K + 1, QW], F32, tag="l")
            p_b = l_psum.tile([K + 1, QW], F32, tag="l")
            for ct in range(CT):
                for p_l, q in ((p_a, q0), (p_b, q1)):
                    nc.tensor.matmul(
                        p_l,
                        lhsT=wm_sb[:, ct],
                        rhs=xts[b][ct][:, ts(q, QW)],
                        start=(ct == 0),
                        stop=(ct == CT - 1),
                    )
            # exp straight from psum; row K is exp(0)=1 (ones row);
            # per-chunk row sums accumulate into sums[:, q]
            for p_l, q in ((p_a, q0), (p_b, q1)):
                nc.scalar.activation(
                    out=attns[b][:, ts(q, QW)], in_=p_l,
                    func=mybir.ActivationFunctionType.Exp,
                    accum_out=sums_t[b][:, q:q + 1],
                )

        def softmax_finish(b):
            total = small.tile([K + 1, 1], F32, tag="total")
            nc.vector.reduce_sum(out=total, in_=sums_t[b],
                                 axis=mybir.AxisListType.X)
            rsum = small.tile([K + 1, 1], F32, tag="rsum")
            nc.vector.reciprocal(out=rsum, in_=total)
            # ones-row (BN shift) must not be normalized
            nc.vector.memset(rsum[K:K + 1], 1.0)
            # fold softmax 1/sum into the mm2 weights: w2m_b = rsum * w2m
            # (ACT per-partition scale; DVE's tensor_scalar here costs 3us)
            nc.scalar.activation(
                out=w2mb_t[b], in_=w2m_sb,
                func=mybir.ActivationFunctionType.Copy, scale=rsum)

        def mm2_ctile(y_psum, b, ct, last=False):
            out_t = out_pool.tile([128, N], BF16, tag="out",
                                  name=f"out{b}_{ct}")
            for q in range(NQ):
                p_y = y_psum.tile([128, QW], F32, tag="y")
                # engine balance: b0's epilogue hides behind b1's mm1 on the
                # PE, so b0 leans on DVE; b1's tail splits PE/ACT vs DVE
                pe_add = q in ((0, 4), (0, 2, 4, 6))[b]
                nc.tensor.matmul(
                    p_y,
                    lhsT=w2mb_t[b][:, ts(ct, 128)],
                    rhs=attns[b][:, ts(q, QW)],
                    start=True, stop=not pe_add,
                )
                if pe_add:
                    # residual add for free on the PE: psum += I @ xT;
                    # relu straight from psum on ACT
                    nc.tensor.matmul(
                        p_y,
                        lhsT=ident_sb,
                        rhs=xts[b][ct][:, ts(q, QW)],
                        start=False, stop=True,
                    )
                    nc.scalar.activation(
                        out=out_t[:, ts(q, QW)], in_=p_y,
                        func=mybir.ActivationFunctionType.Relu,
                    )
                else:
                    # DVE residual add psum+xT -> bf16, then cheap 16-bit
                    # in-place relu (DVE mostly; ACT takes one for balance)
                    nc.vector.scalar_tensor_tensor(
                        out=out_t[:, ts(q, QW)],
                        in0=p_y, scalar=1.0, in1=xts[b][ct][:, ts(q, QW)],
                        op0=mybir.AluOpType.mult, op1=mybir.AluOpType.add,
                    )
                    if q == 7 and b == 0:
                        nc.scalar.activation(
                            out=out_t[:, ts(q, QW)], in_=out_t[:, ts(q, QW)],
                            func=mybir.ActivationFunctionType.Relu,
                        )
                    else:
                        nc.vector.tensor_scalar_max(
                            out_t[:, ts(q, QW)], out_t[:, ts(q, QW)], 0.0)
                if q == NQ // 2 - 1 or q == NQ - 1:
                    # store in halves; the very last half goes out in
                    # quarters so the drain tail after the final relu is
                    # as short as possible
                    lo = 0 if q == NQ // 2 - 1 else N // 2
                    if last and q == NQ - 1:
                        for o in (0, N // 4):
                            nc.gpsimd.dma_start(
                                out=ybs[b][ct][:, lo + o:lo + o + N // 4],
                                in_=out_t[:, lo + o:lo + o + N // 4])
                    else:
                        nc.gpsimd.dma_start(
                            out=ybs[b][ct][:, lo:lo + N // 2],
                            in_=out_t[:, lo:lo + N // 2])

        # preload the ACT exp table off the critical path via a dummy exp
        dummy = small.tile([1, 1], F32, tag="dummy")
        nc.vector.memset(dummy, 0.0)
        nc.scalar.activation(out=dummy, in_=dummy,
                             func=mybir.ActivationFunctionType.Exp)
        # constants first on SP so mm1 can start the moment x arrives
        nc.sync.dma_start(out=wm_sb.rearrange("p ct k -> p (ct k)"),
                          in_=wm[:, :])
        nc.sync.dma_start(out=w2m_sb, in_=w2m[:, :])
        nc.sync.dma_start(out=ident_sb, in_=ident[:, :])
        load_batch(0)
        load_batch(1)

        with tc.tile_pool(name="lps", bufs=4, space="PSUM") as l_psum, \
             tc.tile_pool(name="yps", bufs=4, space="PSUM") as y_psum:
            for qq in range(NQ // 2):
                mm1_pair(l_psum, 0, 2 * qq)
            softmax_finish(0)
            # b1 mm1 interleaved with b0's mm2 epilogue. The first round
            # leads with b0's mm2 (b1's loads haven't landed yet and the
            # in-order PE must not stall on them); later rounds lead with
            # the b1 pair, which fills the PE while DVE/ACT drain the
            # previous mm2 ctile's psum consumers.
            for ct in range(CT):
                mm1_pair(l_psum, 1, 2 * ct)
                if ct == CT - 1:
                    softmax_finish(1)
                mm2_ctile(y_psum, 0, ct)
            for ct in range(CT):
                mm2_ctile(y_psum, 1, ct, last=(ct == CT - 1))

    nc.finalize()
    return nc


def _get_nc() -> bass.Bass:
    global _cached_nc
    if _cached_nc is None:
        _cached_nc = _build_nc()
    return _cached_nc


def _fold_weights(w1, m0, m1, w2, gamma, beta, bn_mean, bn_var):
    w1 = np.asarray(w1, np.float64)
    m0 = np.asarray(m0, np.float64)
    m1 = np.asarray(m1, np.float64)
    w2 = np.asarray(w2, np.float64)
    gamma = np.asarray(gamma, np.float64)
    beta = np.asarray(beta, np.float64)
    bn_mean = np.asarray(bn_mean, np.float64)
    bn_var = np.asarray(bn_var, np.float64)

    wm_aug = np.zeros((C, K + 1), np.float32)
    wm_aug[:, :K] = (w1 @ m0).astype(np.float32)  # col K stays 0 -> ones row
    scale = gamma / np.sqrt(bn_var + BN_EPS)
    w2m_aug = np.zeros((K + 1, C), np.float32)
    w2m_aug[:K] = (m1 @ (w2 * scale[None, :])).astype(np.float32)
    w2m_aug[K] = (beta - bn_mean * scale).astype(np.float32)  # shift row
    return wm_aug, w2m_aug


def _run(inputs_np: dict, trace: bool = False):
    nc = _get_nc()
    inp = np.asarray(inputs_np["inputs"], np.float32)
    wm_aug, w2m_aug = _fold_weights(
        inputs_np["w1"], inputs_np["m0"], inputs_np["m1"], inputs_np["w2"],
        inputs_np["gamma"], inputs_np["beta"],
        inputs_np["bn_mean"], inputs_np["bn_var"],
    )
    # per-batch transposed bf16 inputs: [B, C, N]
    xT = np.ascontiguousarray(
        inp.reshape(B, N, C).astype(BF16_NP).transpose(0, 2, 1))
    # swizzle wm to [p, ct*(K+1)]: row p holds wm[ct*128+p, :] for ct=0..3
    wm_bf = np.ascontiguousarray(
        wm_aug.astype(BF16_NP).reshape(CT, 128, K + 1)
        .transpose(1, 0, 2).reshape(128, CT * (K + 1)))
    eye = np.eye(128, dtype=BF16_NP)
    in_maps = [
        {
            "x": xT[i * BPC:(i + 1) * BPC],
            "wm": wm_bf,
            "w2m": w2m_aug,
            "ident": eye,
        }
        for i in range(NCORES)
    ]
    res = run_bass_kernel_spmd(nc, in_maps, core_ids=list(range(NCORES)),
                               trace=trace)
    out = np.concatenate([r["y"] for r in res.results], axis=0)  # [B, C, N]
    out = out.transpose(0, 2, 1).astype(np.float32)
    return np.ascontiguousarray(out).reshape(B, H, W, C), res


def kernel(**inputs) -> np.ndarray:
    out, _ = _run(inputs, trace=False)
    return out


# revision 42
# speedup vs baseline: 1.1280x; 1.0611x over previous
"""ExternalAttention Trainium2 Bass kernel (transposed bf16 dataflow).

Math (per batch b, with N = H*W = 4096 tokens, C = 512, K = 64):
    x      = inputs @ w1 + b1          [N, C]
    logits = x @ m0                    [N, K]
    attn   = softmax(logits, axis=N)   (trailing L1-norm divides by 1+1e-9 ->
                                        skipped; max-shift skipped: logits O(1))
    y      = attn @ m1 @ w2            [N, C]
    out    = relu(BN_affine(y) + inputs)

Host-side folds (tiny C x C / C x K matrices):
    wm    = [w1 @ m0 | 0]                       [C, K+1]  (b1 @ m0 is constant
            per softmax column -> softmax-invariant, dropped; the zero column
            gives exp(0)=1, a ones-row that injects the BN shift via mm2)
    scale = gamma / sqrt(bn_var + eps); shift = beta - bn_mean * scale
    w2m   = [m1 @ (w2 * scale) ; shift]         [K+1, C]
    => out = relu(colsoftmax(x @ wm) @ w2m + x)

Device dataflow (per core, 2 batches, data-parallel over B=16 on 8 cores).
The host ships x TRANSPOSED per batch (xT [C, N], bf16) so the kernel never
transposes on the PE; everything runs in [c, n] layout and the host
transposes the bf16 output back.
    - mm1:  logitsT[65, n512] = sum_ct wm[ct]^T @ xT[ct], chunk-pairs with
      bank-interleaved psum accumulation
    - ACT exp psum -> attn [65, N] bf16 with accumulated row sums; softmax
      row scale 1/sum is folded into a per-batch bf16 copy of w2m (scaling
      w2m rows == scaling attn rows), applied as an ACT per-partition scale
    - mm2:  deltaT[ct] [128, n512] = w2m_b[:, ct]^T @ attn  (single bf16
      matmul, psum-write-bound ~373ns)
    - residual add: some tiles on the PE (ident matmul accumulate) with ACT
      relu from psum; the rest on DVE (psum+xT -> bf16) with cheap 16-bit
      in-place DVE relu
    - two-batch software pipeline: b0's mm2 epilogue interleaves with b1's
      mm1 so the PE never waits on b1's softmax reduction
Queues: SP carries only the 3 tiny constant loads (the SP engine broadcasts
tile-clock semaphores; bulk DMA issue there stalls every engine), x loads
ride SP+gpsimd split by ctile, stores ride gpsimd.
"""

import os
import sys
from contextlib import ExitStack

import numpy as np
import ml_dtypes

for _p in ("/opt/trn_rl_repo", os.path.expanduser("~/.axon_site/_ro/trn_rl_repo")):
    if os.path.isdir(_p) and _p not in sys.path:
        sys.path.insert(0, _p)

import concourse.bass as bass
import concourse.mybir as mybir
import concourse.tile as tile
from concourse import bacc
from concourse.bass import ts
from concourse.bass_utils import run_bass_kernel_spmd

B, H, W, C, K = 16, 64, 64, 512, 64
N = H * W  # 4096 tokens
BN_EPS = 1e-3
NCORES = 8
BPC = B // NCORES  # batches per core = 2

F32 = mybir.dt.float32
BF16 = mybir.dt.bfloat16
BF16_NP = ml_dtypes.bfloat16

CT = 4          # channel tiles of 128
NQ = 8          # token chunks of 512 per batch
QW = 512        # chunk width

_cached_nc = None


def _build_nc() -> bass.Bass:
    nc = bacc.Bacc(None, target_bir_lowering=False, debug=False)
    x = nc.dram_tensor("x", [BPC, C, N], BF16, kind="ExternalInput")
    # wm pre-swizzled on host to [p, ct*(K+1)] so the load is one contiguous
    # run per partition (128 descriptors instead of 512)
    wm = nc.dram_tensor("wm", [128, CT * (K + 1)], BF16, kind="ExternalInput")
    w2m = nc.dram_tensor("w2m", [K + 1, C], F32, kind="ExternalInput")
    ident = nc.dram_tensor("ident", [128, 128], BF16, kind="ExternalInput")
    y = nc.dram_tensor("y", [BPC, C, N], BF16, kind="ExternalOutput")

    with tile.TileContext(nc) as tc, ExitStack() as ctx:
        const = ctx.enter_context(tc.tile_pool(name="const", bufs=1))
        xt_pool = ctx.enter_context(tc.tile_pool(name="xt", bufs=2 * CT))
        attn_pool = ctx.enter_context(tc.tile_pool(name="attn", bufs=2))
        out_pool = ctx.enter_context(tc.tile_pool(name="out", bufs=2))
        small = ctx.enter_context(tc.tile_pool(name="small", bufs=4))

        wm_sb = const.tile([128, CT, K + 1], BF16)   # wm[ct*128+p, k]
        w2m_sb = const.tile([K + 1, C], F32)         # fp32 master
        ident_sb = const.tile([128, 128], BF16)

        xbs = [x[b].rearrange("(ct p) n -> ct p n", p=128) for b in range(BPC)]
        ybs = [y[b].rearrange("(ct p) n -> ct p n", p=128) for b in range(BPC)]

        xts, attns, sums_t, w2mb_t = [], [], [], []

        def load_batch(b):
            xts.append([xt_pool.tile([128, N], BF16, tag="xt",
                                     name=f"xt{b}_{ct}") for ct in range(CT)])
            # b0 gets a fine-grained front so mm1 can start earlier; issue
            # is split between the SP and gpsimd queues (ct0/1 vs ct2/3)
            if b == 0:
                bounds = (0, 1024, 2048, N)
            else:
                bounds = (0, 2048, N)
            for lo, hi in zip(bounds, bounds[1:]):
                for ct in range(CT):
                    eng = nc.gpsimd if (b == 0 and ct >= 2) else nc.sync
                    eng.dma_start(out=xts[b][ct][:, lo:hi],
                                  in_=xbs[b][ct][:, lo:hi])
            attns.append(attn_pool.tile([K + 1, N], BF16, tag="attn",
                                        name=f"attn{b}"))
            sums_t.append(small.tile([K + 1, NQ], F32, tag="sums",
                                     name=f"sums{b}"))
            w2mb_t.append(small.tile([K + 1, C], BF16, tag="w2mb",
                                     name=f"w2mb{b}"))

        def mm1_pair(l_psum, b, q0):
            # two chunks with bank-interleaved accumulation: consecutive PE
            # instructions alternate psum banks, hiding the RMW drain
            q1 = q0 + 1
            p_a = l_psum.tile([K + 1, QW], F32, tag="l")
            p_b = l_psum.tile([K + 1, QW], F32, tag="l")
            for ct in range(CT):
                for p_l, q in ((p_a, q0), (p_b, q1)):
                    nc.tensor.matmul(
                        p_l,
                        lhsT=wm_sb[:, ct],
                        rhs=xts[b][ct][:, ts(q, QW)],
                        start=(ct == 0),
                        stop=(ct == CT - 1),
                    )
            # exp straight from psum; row K is exp(0)=1 (ones row);
            # per-chunk row sums accumulate into sums[:, q]
            for p_l, q in ((p_a, q0), (p_b, q1)):
                nc.scalar.activation(
                    out=attns[b][:, ts(q, QW)], in_=p_l,
                    func=mybir.ActivationFunctionType.Exp,
                    accum_out=sums_t[b][:, q:q + 1],
                )

        def softmax_finish(b):
            total = small.tile([K + 1, 1], F32, tag="total")
            nc.vector.reduce_sum(out=total, in_=sums_t[b],
                                 axis=mybir.AxisListType.X)
            rsum = small.tile([K + 1, 1], F32, tag="rsum")
            nc.vector.reciprocal(out=rsum, in_=total)
            # ones-row (BN shift) must not be normalized
            nc.vector.memset(rsum[K:K + 1], 1.0)
            # fold softmax 1/sum into the mm2 weights: w2m_b = rsum * w2m
            # (ACT per-partition scale; DVE's tensor_scalar here costs 3us)
            nc.scalar.activation(
                out=w2mb_t[b], in_=w2m_sb,
                func=mybir.ActivationFunctionType.Copy, scale=rsum)

        def mm2_ctile(y_psum, b, ct, last=False):
            out_t = out_pool.tile([128, N], BF16, tag="out",
                                  name=f"out{b}_{ct}")
            for q in range(NQ):
                p_y = y_psum.tile([128, QW], F32, tag="y")
                # engine balance: b0's epilogue hides behind b1's mm1 on the
                # PE, so b0 leans on DVE; b1's tail splits PE/ACT vs DVE
                pe_add = q in ((0, 4), (0, 2, 4, 6))[b]
                nc.tensor.matmul(
                    p_y,
                    lhsT=w2mb_t[b][:, ts(ct, 128)],
                    rhs=attns[b][:, ts(q, QW)],
                    start=True, stop=not pe_add,
                )
                if pe_add:
                    # residual add for free on the PE: psum += I @ xT;
                    # relu straight from psum on ACT
                    nc.tensor.matmul(
                        p_y,
                        lhsT=ident_sb,
                        rhs=xts[b][ct][:, ts(q, QW)],
                        start=False, stop=True,
                    )
                    nc.scalar.activation(
                        out=out_t[:, ts(q, QW)], in_=p_y,
                        func=mybir.ActivationFunctionType.Relu,
                    )
                else:
                    # DVE residual add psum+xT -> bf16, then cheap 16-bit
                    # in-place relu (DVE mostly; ACT takes one for balance)
                    nc.vector.scalar_tensor_tensor(
                        out=out_t[:, ts(q, QW)],
                        in0=p_y, scalar=1.0, in1=xts[b][ct][:, ts(q, QW)],
                        op0=mybir.AluOpType.mult, op1=mybir.AluOpType.add,
                    )
                    if q == 7 and b == 0:
                        nc.scalar.activation(
                            out=out_t[:, ts(q, QW)], in_=out_t[:, ts(q, QW)],
                            func=mybir.ActivationFunctionType.Relu,
                        )
                    else:
                        nc.vector.tensor_scalar_max(
                            out_t[:, ts(q, QW)], out_t[:, ts(q, QW)], 0.0)
                if q == NQ // 2 - 1 or q == NQ - 1:
                    # store in halves so the final store tail is short
                    lo = 0 if q == NQ // 2 - 1 else N // 2
                    nc.gpsimd.dma_start(
                        out=ybs[b][ct][:, lo:lo + N // 2],
                        in_=out_t[:, lo:lo + N // 2])

        # preload the ACT exp table off the critical path via a dummy exp
        dummy = small.tile([1, 1], F32, tag="dummy")
        nc.vector.memset(dummy, 0.0)
        nc.scalar.activation(out=dummy, in_=dummy,
                             func=mybir.ActivationFunctionType.Exp)
        # constants first on SP so mm1 can start the moment x arrives
        nc.sync.dma_start(out=wm_sb.rearrange("p ct k -> p (ct k)"),
                          in_=wm[:, :])
        nc.sync.dma_start(out=w2m_sb, in_=w2m[:, :])
        nc.sync.dma_start(out=ident_sb, in_=ident[:, :])
        load_batch(0)
        load_batch(1)

        with tc.tile_pool(name="lps", bufs=4, space="PSUM") as l_psum, \
             tc.tile_pool(name="yps", bufs=4, space="PSUM") as y_psum:
            for qq in range(NQ // 2):
                mm1_pair(l_psum, 0, 2 * qq)
            softmax_finish(0)
            # b1 mm1 interleaved with b0's mm2 epilogue. The first round
            # leads with b0's mm2 (b1's loads haven't landed yet and the
            # in-order PE must not stall on them); later rounds lead with
            # the b1 pair, which fills the PE while DVE/ACT drain the
            # previous mm2 ctile's psum consumers.
            for ct in range(CT):
                mm1_pair(l_psum, 1, 2 * ct)
                if ct == CT - 1:
                    softmax_finish(1)
                mm2_ctile(y_psum, 0, ct)
            for ct in range(CT):
                mm2_ctile(y_psum, 1, ct, last=(ct == CT - 1))

    nc.finalize()
    return nc


def _get_nc() -> bass.Bass:
    global _cached_nc
    if _cached_nc is None:
        _cached_nc = _build_nc()
    return _cached_nc


def _fold_weights(w1, m0, m1, w2, gamma, beta, bn_mean, bn_var):
    w1 = np.asarray(w1, np.float64)
    m0 = np.asarray(m0, np.float64)
    m1 = np.asarray(m1, np.float64)
    w2 = np.asarray(w2, np.float64)
    gamma = np.asarray(gamma, np.float64)
    beta = np.asarray(beta, np.float64)
    bn_mean = np.asarray(bn_mean, np.float64)
    bn_var = np.asarray(bn_var, np.float64)

    wm_aug = np.zeros((C, K + 1), np.float32)
    wm_aug[:, :K] = (w1 @ m0).astype(np.float32)  # col K stays 0 -> ones row
    scale = gamma / np.sqrt(bn_var + BN_EPS)
    w2m_aug = np.zeros((K + 1, C), np.float32)
    w2m_aug[:K] = (m1 @ (w2 * scale[None, :])).astype(np.float32)
    w2m_aug[K] = (beta - bn_mean * scale).astype(np.float32)  # shift row
    return wm_aug, w2m_aug


def _run(inputs_np: dict, trace: bool = False):
    nc = _get_nc()
    inp = np.asarray(inputs_np["inputs"], np.float32)
    wm_aug, w2m_aug = _fold_weights(
        inputs_np["w1"], inputs_np["m0"], inputs_np["m1"], inputs_np["w2"],
        inputs_np["gamma"], inputs_np["beta"],
        inputs_np["bn_mean"], inputs_np["bn_var"],
    )
    # per-batch transposed bf16 inputs: [B, C, N]
    xT = np.ascontiguousarray(
        inp.reshape(B, N, C).astype(BF16_NP).transpose(0, 2, 1))
    # swizzle wm to [p, ct*(K+1)]: row p holds wm[ct*128+p, :] for ct=0..3
    wm_bf = np.ascontiguousarray(
        wm_aug.astype(BF16_NP).reshape(CT, 128, K + 1)
        .transpose(1, 0, 2).reshape(128, CT * (K + 1)))
    eye = np.eye(128, dtype=BF16_NP)
    in_maps = [
        {
            "x": xT[i * BPC:(i + 1) * BPC],
            "wm": wm_bf,
            "w2m": w2m_aug,
            "ident": eye,
        }
        for i in range(NCORES)
    ]
    res = run_bass_kernel_spmd(nc, in_maps, core_ids=list(range(NCORES)),
                               trace=trace)
    out = np.concatenate([r["y"] for r in res.results], axis=0)  # [B, C, N]
    out = out.transpose(0, 2, 1).astype(np.float32)
    return np.ascontiguousarray(out).reshape(B, H, W, C), res


def kernel(**inputs) -> np.ndarray:
    out, _ = _run(inputs, trace=False)
    return out
